# revision 20
# baseline (speedup 1.0000x reference)
"""NeuralODE forward (Euler, whole-sequence) on 8 Trainium2 NeuronCores.

Math (per step): z <- z + h * (tanh([z, u] @ W1 + b1) @ W2 + b2)
Shapes: z0 [4096, 256], u [4096, 64], W1 [320, 1024], W2 [1024, 256],
t [11]; the fp32 time grid yields 16 Euler micro-steps, 10 recorded.

Sharding: data-parallel over the batch axis, 512 rows per core, split
into two interleaved 256-row waves so the scalar engine (tanh) and the
tensor engine never idle across the per-step serial chain (mm2 -> z8
cast -> mm1) of the other wave.

Device scheme (per core, per wave, all matmuls fp8 e4m3):
  - mm1 z-part: DoubleRow matmuls (K=256 in one instruction, 2 fp8
    moving elements/cycle).  Moving operands are PAIR-INTERLEAVED in
    SBUF ([P, N, 2] + AP rearrange "p n i -> p i n") and weight tiles
    are compact [P, 2, 128] blocks -- measured on HW, this keeps the
    matmul cadence at ~109ns for N=256 (LDWEIGHTS fully hidden),
    where slot-major moving data + strided weight slices ran at 403ns.
  - mm1 u-part: one non-DR K=65 matmul per hidden chunk ([u; 1] @
    [8*W1u; 8*b1]) accumulating onto the z-part PSUM, so b1 and the
    u-contribution ride the PE and tanh reads PSUM directly.
  - The z accumulator lives IN PSUM, scaled: Z = 256*z. mm2 (DoubleRow,
    weights 256*h*W2 quantized) accumulates onto Z with start=False;
    there is no vector-engine z update. Z is initialized by a (256*I)
    @ z0 f32r matmul (start=True sets the PSUM accumulate bits, which
    engine writes would not).
  - tanh: one ACT instruction per 4 hidden chunks ([128,4,256] PSUM
    group, scale=1/8), writing fp8 h8 pair-interleaved for mm2.
  - Per step per wave, DVE does one [128,2,256] cast z8 = Z*(1/256) ->
    fp8 and, on emit steps, one descaled fp32 copy for the output DMA
    (DMA cannot read PSUM).
  - PSUM budget: 2 Z banks + 3x2 group banks = 8 exactly.

Accuracy (numpy sim of this exact quantization scheme): fro rel err
8.1e-3, absmax-rel 1.3e-2 vs the fp32 reference -- inside the 2e-2
gate; robust to flush-to-zero subnormal handling.
"""

import math
import sys

import numpy as np
import ml_dtypes

sys.path.insert(0, "/opt/trn_rl_repo")

import concourse.mybir as mybir
import concourse.tile as tile
from concourse import bacc
from concourse.bass_utils import run_bass_kernel_spmd

H_MAX = 0.05
N_CORES = 8
P = 128
B = 512  # batch rows per core
W = 256  # rows per wave (2 waves per core)
D = 256  # z dim -> KD partition chunks
U = 64  # u dim
H = 1024  # hidden -> KH partition chunks
KD = D // P  # 2
KH = H // P  # 8

A_SC = 8.0  # W1 prescale (tanh applies 1/8)
S_SC = 256.0  # Z = S_SC * z in PSUM; W2h quantized as S_SC*h*W2
E4 = ml_dtypes.float8_e4m3  # == TRN FP8_EXP4 (max 240)

MAX_UNIQUE_H = 8
TRACE = False  # set by test.py to collect a HW profile
TRACE_DIR = None  # set by test.py: directory for NTFF artifacts

_program_cache: dict = {}


def _q8(x, scale=1.0):
    return (np.asarray(x, np.float32) * np.float32(scale)).astype(E4)


def _steps_from_t(t_np):
    """Replicate the reference's trace-time step derivation."""
    steps = []
    for i_t in range(t_np.shape[0] - 1):
        t0f, t1f = float(t_np[i_t]), float(t_np[i_t + 1])
        n_steps = int(math.ceil(abs(t1f - t0f) / H_MAX))
        h = np.float32((t1f - t0f) / n_steps)
        for s in range(n_steps):
            steps.append((float(h), s == n_steps - 1))
    return steps


def _build_program(steps, n_uniq, h_idx, b2_zero):
    f32 = mybir.dt.float32
    f32r = mybir.dt.float32r
    f8 = mybir.dt.float8e4
    DR = mybir.MatmulPerfMode.DoubleRow
    Tanh = mybir.ActivationFunctionType.Tanh
    mult = mybir.AluOpType.mult

    nc = bacc.Bacc(
        "TRN2", target_bir_lowering=False, debug=False, num_devices=N_CORES
    )

    # pk1 = z80 (initial z8, pair-interleaved [p, wave, b, slot] = 1024)
    #     ++ w1q ([m, slot, c] compact DR weight blocks = 2048)
    # packed into one DMA: each dma_start costs ~650ns of serial issue
    # time on the Sync queue, so the step-0-critical inputs ride fewer.
    pk1d = nc.dram_tensor("pk1", [P, 2 * W * 2 + KH * 2 * P], f8,
                          kind="ExternalInput")
    # pk2 = u8aug ([u; ones] per wave [wave, b] = 512) ++ w1uq ([m, c]
    # = [8*W1u; 8*b1] = 1024), both on 65 partitions
    pk2d = nc.dram_tensor("pk2", [U + 1, 2 * W + KH * P], f8,
                          kind="ExternalInput")
    # pk3 = si (256*I, 128) ++ z0r ([wave, n, b] = 1024) in f32r
    pk3d = nc.dram_tensor("pk3", [P, P + 2 * KD * W], f32r,
                          kind="ExternalInput")
    # w2q: [q][p, j, n, slot, c] compact DR weight blocks
    w2qd = nc.dram_tensor(
        "w2q", [n_uniq, P, KH // 2, KD, 2, P], f8, kind="ExternalInput"
    )
    if not b2_zero:
        b2d = nc.dram_tensor("b2q", [1, n_uniq, KD, P], f8, kind="ExternalInput")
        onesd = nc.dram_tensor("ones8", [1, W], f8, kind="ExternalInput")
    n_rec = sum(1 for _, e in steps if e)
    outd = nc.dram_tensor("out", [n_rec, 2, P, KD, W], f32r, kind="ExternalOutput")

    n_steps = len(steps)

    with tile.TileContext(nc) as tc:
        with (
            tc.tile_pool(name="const", bufs=1) as const,
            tc.tile_pool(name="zpool", bufs=2) as zpool,
            tc.tile_pool(name="hpool", bufs=3) as hpool,
            tc.tile_pool(name="empool", bufs=2) as empool,
            tc.tile_pool(name="psum", bufs=1, space="PSUM") as psum,
        ):
            # DMAs ordered by first use: pk2 leads because the first PE
            # work (mm1_u) needs only u8/w1uq; z80/w1q (pk1) are not read
            # until the z-part ~1.5us later.
            pk2_sb = const.tile([U + 1, 2 * W + KH * P], f8)
            nc.sync.dma_start(out=pk2_sb[:], in_=pk2d[:])
            pk1_sb = const.tile([P, 2 * W * 2 + KH * 2 * P], f8)
            nc.sync.dma_start(out=pk1_sb[:], in_=pk1d[:])
            pk3_sb = const.tile([P, P + 2 * KD * W], f32r)
            nc.sync.dma_start(out=pk3_sb[:], in_=pk3d[:])
            z80_sb = pk1_sb[:, : 2 * W * 2].rearrange(
                "p (w b i) -> p w b i", w=2, b=W
            )
            w1q_sb = pk1_sb[:, 2 * W * 2 :].rearrange(
                "p (m s c) -> p m s c", m=KH, s=2
            )
            u8_sb = pk2_sb[:, : 2 * W].rearrange("p (w b) -> p w b", w=2)
            w1u_sb = pk2_sb[:, 2 * W :].rearrange("p (m c) -> p m c", m=KH)
            si_sb = pk3_sb[:, :P]
            z0_sb = pk3_sb[:, P:].rearrange("p (w n b) -> p w n b", w=2, n=KD)
            w2q_sb = const.tile([P, n_uniq, KH // 2, KD, 2, P], f8)
            uniq_order = [h_idx[0]] + [q for q in range(n_uniq) if q != h_idx[0]]
            for q in uniq_order:
                nc.sync.dma_start(out=w2q_sb[:, q], in_=w2qd[q])
            if not b2_zero:
                b2_sb = const.tile([1, n_uniq, KD, P], f8)
                nc.sync.dma_start(out=b2_sb[:], in_=b2d[:])
                ones_sb = const.tile([1, W], f8)
                nc.sync.dma_start(out=ones_sb[:], in_=onesd[:])

            zps = [
                psum.tile([P, KD, W], f32, tag=f"zps{w}", bufs=1, name=f"zps{w}")
                for w in (0, 1)
            ]

            # PE warm-up: the tensor engine idles ~5us waiting for the
            # first input DMA and then ramps its clock over ~3us of busy
            # time (first real matmuls run at ~1.2GHz instead of 2.4GHz).
            # Fill the DMA wait with dummy matmuls on a memset scratch so
            # the ramp happens before the real work arrives.
            scr = const.tile([P, 5 * P], f32r)
            nc.gpsimd.memset(scr[:].bitcast(f32), 0.0)
            for k in range(12):
                gpw = psum.tile([P, 4, W], f32, tag="gps", bufs=3, name="gpw")
                nc.tensor.matmul(
                    gpw[:, 0, :], scr[:, :P], scr[:, P : P + W],
                    start=True, stop=True,
                )

            def mm1_u(w):
                """The u/b1 part of mm1 (needs no z8 -- PE filler while
                the z8 cast runs).  Per 2-bank group: the first matmul
                starts the accumulation group (clearing the whole bank's
                accumulate bits); the second writes its slice with
                start=False (bits unset there -> overwrite); the z-part
                then accumulates onto set bits.  Returns the gps tiles."""
                gps = []
                for g in (0, 1):
                    gp = psum.tile([P, 4, W], f32, tag="gps", bufs=3)
                    for mm in range(4):
                        m = 4 * g + mm
                        nc.tensor.matmul(
                            gp[:, mm, :], w1u_sb[:, m], u8_sb[:, w],
                            start=(mm % 2 == 0), stop=False,
                            skip_group_check=True,
                        )
                    gps.append(gp)
                return gps

            def mm1_z_g(w, zsrc, gps, g):
                """The z part of mm1 (DoubleRow) + tanh for one 4-chunk
                group; zsrc is the DR-moving AP [P, 2, W]."""
                gp = gps[g]
                for mm in range(4):
                    m = 4 * g + mm
                    nc.tensor.matmul(
                        gp[:, mm, :], w1q_sb[:, m], zsrc,
                        start=False, stop=True, perf_mode=DR,
                        skip_group_check=True,
                    )
                # h8 pair-interleaved: [p, j(pair), b, i(slot)]
                h8t = hpool.tile([P, 2, W, 2], f8, tag="h8", bufs=3)
                nc.scalar.activation(
                    h8t[:].rearrange("p j b i -> p j i b"),
                    gp[:].rearrange("p (j i) b -> p j i b", j=2),
                    Tanh, scale=1.0 / A_SC,
                )
                return h8t

            def mm2_group(w, q, h8t, g, with_b2):
                if with_b2 and not b2_zero:
                    for n in range(KD):
                        nc.tensor.matmul(
                            zps[w][:, n, :], b2_sb[0:1, q, n, :], ones_sb[0:1, :],
                            start=False, stop=False, skip_group_check=True,
                        )
                for jj in (0, 1):
                    j = 2 * g + jj
                    mv = h8t[:, jj].rearrange("p b i -> p i b")
                    for n in range(KD):
                        nc.tensor.matmul(
                            zps[w][:, n, :],
                            w2q_sb[:, q, j, n],
                            mv,
                            start=False,
                            stop=(j == KH // 2 - 1),
                            perf_mode=DR,
                            skip_group_check=True,
                        )

            def cast_wave(w):
                z8t = zpool.tile([P, W, 2], f8, tag=f"z8w{w}", bufs=3)
                nc.vector.tensor_scalar(
                    out=z8t[:].rearrange("p b i -> p i b"),
                    in0=zps[w][:],
                    scalar1=1.0 / S_SC, scalar2=None, op0=mult,
                )
                return z8t[:].rearrange("p b i -> p i b")

            def emit_wave(w, rec):
                em = empool.tile([P, KD, W], f32r, tag="em", bufs=3)
                nc.vector.tensor_scalar(
                    out=em[:], in0=zps[w][:],
                    scalar1=1.0 / S_SC, scalar2=None, op0=mult,
                )
                nc.sync.dma_start(out=outd[rec, w], in_=em[:])

            # Prologue: step-0 mm1 from the host-quantized z8, and the Z
            # accumulator init Z = (256*I)^T @ z0 (start=True so the PSUM
            # accumulate bits are set for the whole bank).
            h8_cur = {}
            for w in (0, 1):
                gps0 = mm1_u(w)
                zsrc0 = z80_sb[:, w].rearrange("p b i -> p i b")
                h8_cur[w] = [
                    mm1_z_g(w, zsrc0, gps0, 0),
                    mm1_z_g(w, zsrc0, gps0, 1),
                ]
            # Z init after both waves' prologue mm1 so the si/z0r DMAs
            # overlap compute instead of blocking the PE queue.
            for w in (0, 1):
                nc.tensor.matmul(
                    zps[w][:, 0, :], si_sb, z0_sb[:, w, 0, :],
                    start=True, stop=False,
                )
                nc.tensor.matmul(
                    zps[w][:, 1, :], si_sb, z0_sb[:, w, 1, :],
                    start=False, stop=True,
                )

            # Software-pipelined loop, emission in PE ready-time order so
            # the in-order engine queues never head-of-line block: the
            # u-matmuls of the NEXT step fill the PE while the z8 cast
            # runs on DVE; each wave's serial chain (mm2 -> cast -> z-mm1
            # -> tanh) hides under the other wave's work.
            rec = 0
            for s, (h_i, emit) in enumerate(steps):
                q = h_idx[s]
                last = s == n_steps - 1
                mm2_group(0, q, h8_cur[0][0], 0, True)
                if not last:
                    gps_w0 = mm1_u(0)
                mm2_group(0, q, h8_cur[0][1], 1, False)
                if not last:
                    z8_w0 = cast_wave(0)
                if last and emit:
                    emit_wave(0, rec)
                mm2_group(1, q, h8_cur[1][0], 0, True)
                if not last:
                    h8n_w0g0 = mm1_z_g(0, z8_w0, gps_w0, 0)
                mm2_group(1, q, h8_cur[1][1], 1, False)
                if not last:
                    h8n_w0g1 = mm1_z_g(0, z8_w0, gps_w0, 1)
                    h8_cur[0] = [h8n_w0g0, h8n_w0g1]
                    z8_w1 = cast_wave(1)
                    gps_w1 = mm1_u(1)
                if emit:
                    if not last:
                        emit_wave(0, rec)
                    emit_wave(1, rec)
                    rec += 1
                if not last:
                    h8_cur[1] = [
                        mm1_z_g(1, z8_w1, gps_w1, 0),
                        mm1_z_g(1, z8_w1, gps_w1, 1),
                    ]

    nc.compile()
    return nc


def kernel(z0, u, t, W1, b1, W2, b2):
    z0 = np.ascontiguousarray(np.asarray(z0, dtype=np.float32))
    u = np.ascontiguousarray(np.asarray(u, dtype=np.float32))
    t_np = np.asarray(t, dtype=np.float32)
    W1 = np.ascontiguousarray(np.asarray(W1, dtype=np.float32))
    b1 = np.ascontiguousarray(np.asarray(b1, dtype=np.float32))
    W2 = np.ascontiguousarray(np.asarray(W2, dtype=np.float32))
    b2 = np.ascontiguousarray(np.asarray(b2, dtype=np.float32))

    bs, dim = z0.shape
    assert (bs, dim) == (N_CORES * B, D), (bs, dim)
    assert u.shape == (bs, U) and W1.shape == (D + U, H)
    assert W2.shape == (H, D) and b1.shape == (H,) and b2.shape == (D,)

    steps = _steps_from_t(t_np)
    n_rec = sum(1 for _, e in steps if e)
    if n_rec == 0:
        return z0[None].copy()

    uniq_h = sorted(set(h for h, _ in steps))
    assert len(uniq_h) <= MAX_UNIQUE_H, (
        f"{len(uniq_h)} unique step sizes; raise MAX_UNIQUE_H"
    )
    h_idx = [uniq_h.index(h) for h, _ in steps]
    n_uniq = len(uniq_h)
    b2_zero = bool(np.all(b2 == 0.0))

    key = (tuple(steps), n_uniq, tuple(h_idx), b2_zero)
    nc = _program_cache.get(key)
    if nc is None:
        nc = _build_program(steps, n_uniq, h_idx, b2_zero)
        _program_cache[key] = nc

    # Shared (replicated) weight packs.
    # w1q[p,m,i,c] = q8(8*W1[i*128+p, 128m+c])
    w1q = np.ascontiguousarray(
        _q8(W1[:D].reshape(KD, P, KH, P).transpose(1, 2, 0, 3), A_SC)
    )
    # w1uq[p,m,c]: rows 0..63 = 8*W1u, row 64 = 8*b1 (ones-row bias)
    w1uq = np.zeros((U + 1, KH, P), dtype=E4)
    w1uq[:U] = _q8(W1[D:].reshape(U, KH, P), A_SC)
    w1uq[U] = _q8(b1.reshape(KH, P), A_SC)
    w1uq = np.ascontiguousarray(w1uq)
    # w2q[q,p,j,n,i,c] = q8(256*h_q*W2[128*(2j+i)+p, 128n+c])
    w2r = W2.reshape(KH // 2, 2, P, KD, P)
    w2q = np.ascontiguousarray(
        np.stack([_q8(w2r, S_SC * h).transpose(2, 0, 3, 1, 4) for h in uniq_h])
    )
    si = np.ascontiguousarray(np.eye(P, dtype=np.float32) * np.float32(S_SC))
    if not b2_zero:
        b2q = np.ascontiguousarray(
            np.stack([_q8(b2.reshape(KD, P), S_SC * h) for h in uniq_h])[None]
        )
        ones8 = np.ones((1, W), dtype=E4)

    in_maps = []
    for c in range(N_CORES):
        sl = slice(c * B, (c + 1) * B)
        # z0r[p,w,n,b] = z0[c*512 + w*256 + b, n*128+p]
        z0r = np.ascontiguousarray(
            z0[sl].reshape(2, W, KD, P).transpose(3, 0, 2, 1)
        )
        # z80[p,w,b,i] pair-interleaved initial z8
        z80 = np.ascontiguousarray(_q8(z0r.transpose(0, 1, 3, 2)))
        u8c = np.ones((U + 1, 2, W), dtype=E4)
        u8c[:U] = _q8(u[sl].reshape(2, W, U).transpose(2, 0, 1))
        m = {
            "pk1": np.ascontiguousarray(
                np.concatenate([z80.reshape(P, -1), w1q.reshape(P, -1)], 1)
            ),
            "pk2": np.ascontiguousarray(
                np.concatenate(
                    [u8c.reshape(U + 1, -1), w1uq.reshape(U + 1, -1)], 1
                )
            ),
            "pk3": np.ascontiguousarray(
                np.concatenate([si, z0r.reshape(P, -1)], 1)
            ),
            "w2q": w2q,
        }
        if not b2_zero:
            m["b2q"] = b2q
            m["ones8"] = ones8
        in_maps.append(m)

    res = run_bass_kernel_spmd(
        nc, in_maps, list(range(N_CORES)), trace=TRACE, tmpdir=TRACE_DIR
    )
    kernel.last_results = res

    full = np.empty((n_rec + 1, bs, dim), dtype=np.float32)
    full[0] = z0
    for c in range(N_CORES):
        o = np.asarray(res.results[c]["out"], dtype=np.float32)
        # o[r,w,p,n,b] -> full[1+r, c*512 + w*256 + b, n*128 + p]
        full[1:, c * B : (c + 1) * B, :] = o.transpose(0, 1, 4, 3, 2).reshape(
            n_rec, B, D
        )
    return full
